# revision 47
# baseline (speedup 1.0000x reference)
"""Trainium2 Bass kernel for nn_Attention: batched small-N attention.

Reference computation (per batch b of 8192, tokens N=17, C=512, H=8 heads, HD=64):
    qkv = x @ W_qkv + b_qkv
    q,k,v split/reshaped; logits = (q @ k^T + alpha*outer)*scale; A = softmax
    out = (A @ v reshaped back) @ W_proj + b_proj

Strategy: pure data parallel over B across 8 cores (1024 batches/core).
Per core, batches are packed into groups of 7 (119 tokens <= 128 partitions) and
macro-tiles of 4 groups (476 tokens) so every big matmul runs with free dim >= 256
in float32r (1 cyc/row). Cross-batch attention inside a group is killed with a
MULTIPLICATIVE mask exp(mask) (0 off-block, exp(alpha*outer*scale) on-block)
applied by the same DVE op that accumulates softmax denominators.

x is shipped as a bf16 hi/lo pair and loaded TRANSPOSED by the DMA xbar
(dma_start_transpose needs 2-byte dtype); gpsimd reconstructs fp32 x^T, so the
tensor engine never runs x transposes and the vector engine never evacuates them.

All biases are folded host-side:
  - scale into W_q/b_q
  - b_qkv (q/k parts) added during the PSUM->SBUF evacuation on the Scalar
    engine (per-partition bias ride-along)
  - b_v and b_proj into one effective output bias: beff = b_proj + b_v @ W_proj
  - alpha*outer*scale into the multiplicative mask
Softmax needs no max-subtraction: |logits| <= ~1 by construction of the inputs.
"""

import numpy as np

B, N, C, H, HD = 8192, 17, 512, 8, 64
NCORES = 8
BC = B // NCORES            # batches per core
NT = BC * N                 # tokens per core
G = 7                       # batches per group
TG = G * N                  # 119 tokens per group
GPM = 4                     # groups per (full) macro tile

# 1024 = 36 * 28 + 16;  final macro = groups of (7, 7, 2) batches
MACROS = [(m * (G * GPM), [7, 7, 7, 7]) for m in range(36)] + [(1008, [7, 7, 2])]

_CACHE = {}

# runtime-selected implementation details (validated on HW by probes)
CFG = {}


def _build_program(macros=None, cfg=None):
    cfg = dict(cfg or {})
    for k, v in dict(xp=3, qkp=2, attp=3, etsp=3, io=6,
                     vq=3, s=2, et=1, av=2,
                     qk_dve=0, attT_dve=1, vg_dve=0, xadd="dve", ygp=1, ets_act=0, xprio=0,
                     dmat=1, tilepos=1, ttr=0, gpadd=1).items():
        cfg.setdefault(k, v)
    import concourse.bass as bass
    import concourse.mybir as mybir
    import concourse.tile as tile
    from concourse import bacc
    from concourse.alu_op_type import AluOpType
    from concourse.masks import make_identity

    f32 = mybir.dt.float32
    f32r = mybir.dt.float32r
    bf16 = mybir.dt.bfloat16
    Act = mybir.ActivationFunctionType

    nc = bacc.Bacc("TRN2", target_bir_lowering=False, debug=False,
                   num_devices=NCORES)

    if cfg["dmat"]:
        xhi_d = nc.dram_tensor("xhi", [NT, C], bf16, kind="ExternalInput")
        xlo_d = nc.dram_tensor("xlo", [NT, C], bf16, kind="ExternalInput")
    else:
        x_d = nc.dram_tensor("x", [NT, C], f32r, kind="ExternalInput")
    wqk_d = nc.dram_tensor("wqk", [C, 2 * C], f32r, kind="ExternalInput")
    wv_d = nc.dram_tensor("wv", [C, C], f32r, kind="ExternalInput")
    wp_d = nc.dram_tensor("wp", [C, C], f32r, kind="ExternalInput")
    bqk_d = nc.dram_tensor("bqk", [2 * C], f32, kind="ExternalInput")
    beff_d = nc.dram_tensor("beff", [1, C], f32, kind="ExternalInput")
    mexp_d = nc.dram_tensor("mexp", [H, TG, TG], bf16, kind="ExternalInput")
    y_d = nc.dram_tensor("y", [NT, C], f32, kind="ExternalOutput")

    with tile.TileContext(nc) as tc:
        with (
            tc.tile_pool(name="stat", bufs=1) as stat,
            tc.tile_pool(name="xp", bufs=cfg["xp"]) as xp,
            tc.tile_pool(name="qkp", bufs=cfg["qkp"]) as qkp,
            tc.tile_pool(name="attp", bufs=cfg["attp"]) as attp,
            tc.tile_pool(name="etsp", bufs=cfg["etsp"]) as etsp,
            tc.tile_pool(name="io", bufs=cfg["io"]) as io,
            tc.tile_pool(name="ps_vq", bufs=cfg["vq"], space="PSUM") as ps_vq,
            tc.tile_pool(name="ps_s", bufs=cfg["s"], space="PSUM") as ps_s,
            tc.tile_pool(name="ps_et", bufs=cfg["et"], space="PSUM") as ps_et,
            tc.tile_pool(name="ps_av", bufs=cfg["av"], space="PSUM") as ps_av,
        ):
            mlist = MACROS if macros is None else macros

            def macro_params(b0, gsizes):
                tgs = [g * N for g in gsizes]
                goffs = np.concatenate([[0], np.cumsum(tgs)]).tolist()
                Tm = goffs[-1]
                t0 = b0 * N
                # transposed-DMA row count: multiple of 16 (476->480; 272 ok)
                rows = Tm if Tm % 16 == 0 else min(-(-Tm // 16) * 16, NT - t0)
                return tgs, goffs, Tm, t0, rows

            def load_x(t0, rows, tgs, goffs):
                if cfg["dmat"]:
                    xhi_t = xp.tile([128, 4, 480], bf16, tag="xhi")
                    xlo_t = xp.tile([128, 4, 480], bf16, tag="xlo")
                    for c in range(4):
                        nc.sync.dma_start_transpose(
                            out=xhi_t[:, c, :rows],
                            in_=xhi_d[t0:t0 + rows, c * 128:(c + 1) * 128])
                        nc.sync.dma_start_transpose(
                            out=xlo_t[:, c, :rows],
                            in_=xlo_d[t0:t0 + rows, c * 128:(c + 1) * 128])
                    return xhi_t, xlo_t
                xgs = []
                for gi in range(len(tgs)):
                    tg, go = tgs[gi], goffs[gi]
                    xg = io.tile([TG, C], f32, tag=f"xg{gi}")
                    nc.sync.dma_start(out=xg[:tg, :],
                                      in_=x_d[t0 + go:t0 + go + tg, :])
                    xgs.append(xg)
                return xgs

            # macro 0's x DMAs first so they are at the head of the DMA queue
            _p0 = macro_params(*mlist[0])
            xpair0 = load_x(_p0[3], _p0[4], _p0[0], _p0[1])

            # ---- static weights: consolidated SWDGE loads (gpsimd queue,
            # no HWDGE contention with the x/y stream), ordered by first use
            wv_all = stat.tile([128, 4, C], f32r, tag="wv")
            nc.gpsimd.dma_start(out=wv_all,
                                in_=wv_d.rearrange("(c p) n -> p c n", c=4))
            wv_sb = [wv_all[:, c, :] for c in range(4)]
            wqk_all = stat.tile([128, 4, 2 * C], f32r, tag="wqk")
            nc.gpsimd.dma_start(out=wqk_all,
                                in_=wqk_d.rearrange("(c p) n -> p c n", c=4))
            wqk_sb = [wqk_all[:, c, :] for c in range(4)]
            bqk_sb = stat.tile([128, 8], f32, tag="bqk")
            nc.gpsimd.dma_start(out=bqk_sb,
                                in_=bqk_d.rearrange("(m p) -> p m", m=8))
            mexp_all = stat.tile([TG, H, TG], bf16, tag="mexp")
            nc.gpsimd.dma_start(out=mexp_all,
                                in_=mexp_d.rearrange("h p n -> p h n"))
            mexp_sb = [mexp_all[:, h, :] for h in range(H)]
            idh = stat.tile([128, 128], bf16, tag="idh")
            make_identity(nc, idh)
            if not cfg["dmat"]:
                idf = stat.tile([128, 128], f32, tag="idf")
                make_identity(nc, idf)
            wp_all = stat.tile([128, 4, C], f32r, tag="wp")
            nc.gpsimd.dma_start(out=wp_all,
                                in_=wp_d.rearrange("(c p) n -> p c n", c=4))
            wp_sb = [wp_all[:, c, :] for c in range(4)]
            beff_sb = stat.tile([128, C], f32, tag="beff")
            nc.gpsimd.dma_start(out=beff_sb,
                                in_=beff_d[0:1, :].partition_broadcast(128))

            for mi, (b0, gsizes) in enumerate(mlist):
                ng = len(gsizes)
                tgs, goffs, Tm, t0, rows = macro_params(b0, gsizes)
                if mi == 0:
                    xpair = xpair0
                else:
                    xpair = load_x(t0, rows, tgs, goffs)
                xT = xp.tile([128, 4, 480], f32r, tag="xT")
                if cfg["dmat"]:
                    xhi_t, xlo_t = xpair
                    # gpsimd adds in f32 (its f32r rounding is broken on HW),
                    # then a gpsimd cast DMA relabels to f32r for the matmuls
                    # (raw f32 bits as f32r -- same as the weight loads).
                    if cfg["xadd"] == "dve":
                        with tc.high_priority(offset=cfg["xprio"] or None):
                            for c in range(4):
                                nc.vector.tensor_tensor(out=xT[:, c, :Tm],
                                                        in0=xhi_t[:, c, :Tm],
                                                        in1=xlo_t[:, c, :Tm],
                                                        op=AluOpType.add)
                    else:
                        x32 = xp.tile([128, 4, 480], f32, tag="x32")
                        eng = nc.vector if mi < 2 else nc.gpsimd
                        for c in range(4):
                            eng.tensor_tensor(out=x32[:, c, :Tm],
                                              in0=xhi_t[:, c, :Tm],
                                              in1=xlo_t[:, c, :Tm],
                                              op=AluOpType.add)
                        nc.gpsimd.dma_start(out=xT[:, :, :Tm],
                                            in_=x32[:, :, :Tm])
                else:
                    for gi in range(ng):
                        tg, go = tgs[gi], goffs[gi]
                        tps = ps_et.tile([128, 4, 128], f32, tag="xt")
                        for c in range(4):
                            nc.tensor.matmul(tps[:, c, :tg],
                                             xpair[gi][:tg, c * 128:(c + 1) * 128],
                                             idf[:tg, :tg], is_transpose=True,
                                             start=True, stop=True)
                        nc.vector.tensor_copy(out=xT[:, :, go:go + tg],
                                              in_=tps[:, :, :tg])

                # ---- v projection per group ----
                v_tiles = []
                for gi in range(ng):
                    tg, go = tgs[gi], goffs[gi]
                    vps = ps_vq.tile([TG, C], f32, tag="vq")
                    for c in range(4):
                        nc.tensor.matmul(vps[:tg, :], xT[:, c, go:go + tg],
                                         wv_sb[c], start=(c == 0), stop=(c == 3))
                    vg = io.tile([TG, C], bf16, tag=f"vg{gi}")
                    if cfg["vg_dve"]:
                        nc.vector.tensor_copy(out=vg[:tg, :], in_=vps[:tg, :])
                    else:
                        nc.scalar.activation(out=vg[:tg, :], in_=vps[:tg, :],
                                             func=Act.Copy)
                    v_tiles.append(vg)

                # ---- qk projection: qkT chunks [128, Tm] (rows of [q;k]) ----
                qkT = []
                for m in range(8):
                    qps = ps_vq.tile([128, 512], f32, tag="vq")
                    for c in range(4):
                        nc.tensor.matmul(qps[:, :Tm],
                                         wqk_sb[c][:, m * 128:(m + 1) * 128],
                                         xT[:, c, :Tm],
                                         start=(c == 0), stop=(c == 3))
                    qk = qkp.tile([128, Tm], bf16, tag=f"qkT{m}")
                    if m < cfg["qk_dve"]:
                        nc.vector.tensor_scalar_add(qk[:, :], qps[:, :Tm],
                                                    bqk_sb[:, m:m + 1])
                    else:
                        nc.scalar.activation(out=qk[:, :], in_=qps[:, :Tm],
                                             func=Act.Identity,
                                             bias=bqk_sb[:, m:m + 1])
                    qkT.append(qk)

                # ---- attention, head pairs ----
                z = attp.tile([TG, 32], f32, tag="z")
                if gsizes != [7] * GPM:
                    nc.vector.memset(z, 1.0)
                rzb = attp.tile([TG, 32], f32, tag="rzb")
                for p in range(4):
                    avps = ps_av.tile([128, 512], f32, tag="av")
                    for h in (2 * p, 2 * p + 1):
                        ro = (h % 2) * 64
                        sps = ps_s.tile([TG, 4, 128], f32, tag="s")
                        for gi in range(ng):
                            tg, go = tgs[gi], goffs[gi]
                            nc.tensor.matmul(sps[:tg, gi, :tg],
                                             qkT[p][ro:ro + 64, go:go + tg],
                                             qkT[4 + p][ro:ro + 64, go:go + tg],
                                             start=True, stop=True)
                        er = attp.tile([TG, ng, TG], bf16, tag="er")
                        if gsizes == [7] * GPM:
                            nc.scalar.activation(out=er[:, :, :],
                                                 in_=sps[:, :, :TG],
                                                 func=Act.Exp)
                        else:
                            for gi in range(ng):
                                tg = tgs[gi]
                                nc.scalar.activation(
                                    out=er[:tg, gi, :tg],
                                    in_=sps[:tg, gi, :tg], func=Act.Exp)
                        em = attp.tile([TG, ng, TG], bf16, tag="em")
                        for gi in range(ng):
                            tg = tgs[gi]
                            if cfg["ttr"] == 1:
                                nc.vector.tensor_tensor_reduce(
                                    out=em[:tg, gi, :tg], in0=er[:tg, gi, :tg],
                                    in1=mexp_sb[h][:tg, :tg], scale=1.0,
                                    scalar=0.0, op0=AluOpType.mult,
                                    op1=AluOpType.add,
                                    accum_out=z[:tg, h * 4 + gi: h * 4 + gi + 1])
                            elif cfg["ttr"] == 2:
                                nc.vector.tensor_tensor(
                                    out=em[:tg, gi, :tg], in0=er[:tg, gi, :tg],
                                    in1=mexp_sb[h][:tg, :tg],
                                    op=AluOpType.mult)
                                nc.vector.tensor_reduce(
                                    out=z[:tg, h * 4 + gi: h * 4 + gi + 1],
                                    in_=em[:tg, gi, :tg],
                                    axis=mybir.AxisListType.X,
                                    op=AluOpType.add)
                            else:
                                nc.vector.scalar_tensor_tensor(
                                    out=em[:tg, gi, :tg],
                                    in0=er[:tg, gi, :tg], scalar=1.0,
                                    in1=mexp_sb[h][:tg, :tg],
                                    op0=AluOpType.mult, op1=AluOpType.mult,
                                    accum_out=z[:tg, h * 4 + gi: h * 4 + gi + 1])
                        nc.vector.reciprocal(out=rzb[:, h * 4:h * 4 + ng],
                                             in_=z[:, h * 4:h * 4 + ng])
                        en = attp.tile([TG, ng, TG], bf16, tag="en")
                        for gi in range(ng):
                            tg = tgs[gi]
                            nc.vector.tensor_scalar_mul(
                                en[:tg, gi, :tg], em[:tg, gi, :tg],
                                rzb[:tg, h * 4 + gi: h * 4 + gi + 1])
                        etps = ps_et.tile([128, 4, 256], bf16, tag="et")
                        for gi in range(ng):
                            tg = tgs[gi]
                            nc.tensor.matmul(etps[:tg, gi, :tg],
                                             en[:tg, gi, :tg], idh[:tg, :tg],
                                             is_transpose=True, start=True,
                                             stop=True)
                        ets = etsp.tile([128, ng, TG + 1], bf16, tag="ets")
                        etseng = nc.scalar if cfg["ets_act"] else nc.vector
                        if gsizes == [7] * GPM:
                            if cfg["ets_act"]:
                                nc.scalar.activation(out=ets[:TG, :, :TG],
                                                     in_=etps[:TG, :GPM, :TG],
                                                     func=Act.Copy)
                            else:
                                nc.vector.tensor_copy(out=ets[:TG, :, :TG],
                                                      in_=etps[:TG, :GPM, :TG])
                        else:
                            for gi in range(ng):
                                tg = tgs[gi]
                                if cfg["ets_act"]:
                                    nc.scalar.activation(
                                        out=ets[:tg, gi, :tg],
                                        in_=etps[:tg, gi, :tg], func=Act.Copy)
                                else:
                                    nc.vector.tensor_copy(
                                        out=ets[:tg, gi, :tg],
                                        in_=etps[:tg, gi, :tg])
                        for gi in range(ng):
                            tg, go = tgs[gi], goffs[gi]
                            nc.tensor.matmul(
                                avps[ro:ro + 64, go:go + tg],
                                v_tiles[gi][:tg, h * 64:(h + 1) * 64],
                                ets[:tg, gi, :tg],
                                tile_position=(0, ro) if cfg["tilepos"] else None,
                                start=True, stop=True)
                    attT = qkp.tile([128, Tm], f32r, tag=f"attT{p}")
                    # DVE: must produce true f32r for the proj matmul
                    nc.vector.tensor_copy(out=attT[:, :], in_=avps[:, :Tm])
                    qkT.append(attT)  # keep alive; index 8+p

                # ---- output projection ----
                for gi in range(ng):
                    tg, go = tgs[gi], goffs[gi]
                    fps = ps_av.tile([TG, C], f32, tag="av")
                    for c in range(4):
                        nc.tensor.matmul(fps[:tg, :], qkT[8 + c][:, go:go + tg],
                                         wp_sb[c], start=(c == 0), stop=(c == 3))
                    yg = io.tile([TG, C], f32, tag="yg")
                    if cfg["ygp"]:
                        yr = io.tile([TG, C], f32, tag="yr")
                        nc.scalar.activation(out=yr[:tg, :], in_=fps[:tg, :],
                                             func=Act.Copy)
                        nc.gpsimd.tensor_tensor(out=yg[:tg, :],
                                                in0=yr[:tg, :],
                                                in1=beff_sb[:tg, :],
                                                op=AluOpType.add)
                    else:
                        nc.vector.tensor_tensor(out=yg[:tg, :],
                                                in0=fps[:tg, :],
                                                in1=beff_sb[:tg, :],
                                                op=AluOpType.add)
                    nc.sync.dma_start(out=y_d[t0 + go: t0 + go + tg, :],
                                      in_=yg[:tg, :])

    nc.compile()
    return nc


def _host_prep(x, W_qkv, b_qkv, outer, alpha, W_proj, b_proj):
    import ml_dtypes

    bf16 = ml_dtypes.bfloat16
    scale = np.float32(HD ** -0.5)
    x = np.ascontiguousarray(np.asarray(x, dtype=np.float32))
    W_qkv = np.asarray(W_qkv, dtype=np.float32)
    b_qkv = np.asarray(b_qkv, dtype=np.float32)
    outer = np.asarray(outer, dtype=np.float32)
    alpha = np.asarray(alpha, dtype=np.float32)
    W_proj = np.asarray(W_proj, dtype=np.float32)
    b_proj = np.asarray(b_proj, dtype=np.float32)

    xhi = x.astype(bf16)
    xlo = (x - xhi.astype(np.float32)).astype(bf16)

    wqk = np.concatenate([W_qkv[:, :C] * scale, W_qkv[:, C:2 * C]], axis=1)
    bqk = np.concatenate([b_qkv[:C] * scale, b_qkv[C:2 * C]])
    wv = W_qkv[:, 2 * C:]
    bv = b_qkv[2 * C:]
    beff = (b_proj + bv @ W_proj)[None, :]

    # multiplicative mask: 0 off-block, exp(alpha*outer*scale) on diag blocks
    base = np.zeros((H, TG, TG), dtype=np.float32)
    bias = np.exp((alpha[0] * scale) * outer)  # [H, 17, 17]
    for i in range(G):
        base[:, i * N:(i + 1) * N, i * N:(i + 1) * N] = bias
    mexp = base.astype(bf16)

    shared = {
        "wqk": np.ascontiguousarray(wqk),
        "wv": np.ascontiguousarray(wv),
        "wp": np.ascontiguousarray(W_proj),
        "bqk": np.ascontiguousarray(bqk),
        "beff": np.ascontiguousarray(beff),
        "mexp": np.ascontiguousarray(mexp),
    }
    return xhi, xlo, shared


def kernel(x, W_qkv, b_qkv, outer, alpha, W_proj, b_proj, _trace=False):
    from concourse.bass_utils import run_bass_kernel_spmd

    if "nc" not in _CACHE:
        _CACHE["nc"] = _build_program(cfg=CFG)
    nc = _CACHE["nc"]

    xhi, xlo, shared = _host_prep(x, W_qkv, b_qkv, outer, alpha, W_proj, b_proj)
    use_dmat = CFG.get("dmat", 1)
    in_maps = []
    for c in range(NCORES):
        m = dict(shared)
        if use_dmat:
            m["xhi"] = np.ascontiguousarray(
                xhi[c * BC:(c + 1) * BC].reshape(NT, C))
            m["xlo"] = np.ascontiguousarray(
                xlo[c * BC:(c + 1) * BC].reshape(NT, C))
        else:
            m["x"] = np.ascontiguousarray(
                np.asarray(x, dtype=np.float32)[c * BC:(c + 1) * BC]
                .reshape(NT, C))
        in_maps.append(m)

    res = run_bass_kernel_spmd(nc, in_maps, core_ids=list(range(NCORES)),
                               trace=_trace)
    out = np.concatenate(
        [res.results[c]["y"].reshape(BC, N, C) for c in range(NCORES)], axis=0)
    if _trace:
        _CACHE["last_result"] = res
    return out


# revision 49
# speedup vs baseline: 1.0519x; 1.0519x over previous
"""Trainium2 Bass kernel for nn_Attention: batched small-N attention.

Reference computation (per batch b of 8192, tokens N=17, C=512, H=8 heads, HD=64):
    qkv = x @ W_qkv + b_qkv
    q,k,v split/reshaped; logits = (q @ k^T + alpha*outer)*scale; A = softmax
    out = (A @ v reshaped back) @ W_proj + b_proj

Strategy: pure data parallel over B across 8 cores (1024 batches/core).
Per core, batches are packed into groups of 7 (119 tokens <= 128 partitions) and
macro-tiles of 4 groups (476 tokens) so every big matmul runs with free dim >=
256 in float32r (1 cyc/row); attention-internal matmuls run in bf16.

Engine choreography (the v1 kernel was DVE-bound at 90%; this one holds PE at
~90% with DVE/ACT at ~80/70%):
  - x is shipped as a bf16 hi/lo pair and loaded TRANSPOSED by the DMA xbar
    (dma_start_transpose, 2-byte dtypes only); the DVE re-adds hi+lo to f32r.
    The tensor engine never runs x transposes.
  - qk / v PSUM evacuations ride the Scalar engine (activation Copy/Identity,
    with the folded q/k bias as the per-partition Identity bias).
  - Cross-batch attention inside a group is killed with a MULTIPLICATIVE mask
    exp(mask) (0 off-block, exp(alpha*outer*scale) on-block) fused into one
    scalar_tensor_tensor per (head, group) whose accum_out also yields the
    softmax denominators z (per-query-partition, so 1/z is a cheap
    per-partition tensor_scalar).  exp itself reads S straight out of PSUM on
    the Scalar engine with no bias add.
  - A^T transposes run on PE (bf16 identity matmuls); A^T@V runs as col-tiled
    (tile_position) head pairs writing both heads into one PSUM tile that IS
    the attT layout chunk for the output projection.
  - output bias add runs on GpSimd after an ACT evacuation.
HW-validated quirks: tensor_tensor_reduce crashes the device (use
scalar_tensor_tensor+accum_out); gpsimd mis-rounds float32r outputs and the
BIR verifier requires f32r matmul operands to be produced as f32r, so every
f32r elementwise producer is the DVE; PSUM tiles must be 2KB bank-aligned.

All biases are folded host-side:
  - scale into W_q/b_q
  - b_qkv (q/k parts) added during the qk PSUM->SBUF evacuation
  - b_v and b_proj into one effective output bias: beff = b_proj + b_v @ W_proj
  - alpha*outer*scale into the multiplicative mask
Softmax needs no max-subtraction: |logits| <= ~1 by construction of the inputs.
"""

import numpy as np

B, N, C, H, HD = 8192, 17, 512, 8, 64
NCORES = 8
BC = B // NCORES            # batches per core
NT = BC * N                 # tokens per core
G = 7                       # batches per group
TG = G * N                  # 119 tokens per group
GPM = 4                     # groups per (full) macro tile

# 1024 = 36 * 28 + 16;  final macro = groups of (7, 7, 2) batches
MACROS = [(m * (G * GPM), [7, 7, 7, 7]) for m in range(36)] + [(1008, [7, 7, 2])]

_CACHE = {}

# runtime-selected implementation details (validated on HW by probes)
CFG = {}


def _build_program(macros=None, cfg=None):
    cfg = dict(cfg or {})
    for k, v in dict(xp=2, qkp=2, attp=2, etsp=3, io=8,
                     vq=4, s=1, et=2, av=1,
                     qk_dve=0, attT_dve=1, vg_dve=0, xadd="dve", ygp=1,
                     ets_act=0, xprio=0,
                     dmat=1, tilepos=1, ttr=0, gpadd=1).items():
        cfg.setdefault(k, v)
    import concourse.bass as bass
    import concourse.mybir as mybir
    import concourse.tile as tile
    from concourse import bacc
    from concourse.alu_op_type import AluOpType
    from concourse.masks import make_identity

    f32 = mybir.dt.float32
    f32r = mybir.dt.float32r
    bf16 = mybir.dt.bfloat16
    Act = mybir.ActivationFunctionType

    nc = bacc.Bacc("TRN2", target_bir_lowering=False, debug=False,
                   num_devices=NCORES)

    if cfg["dmat"]:
        xhi_d = nc.dram_tensor("xhi", [NT, C], bf16, kind="ExternalInput")
        xlo_d = nc.dram_tensor("xlo", [NT, C], bf16, kind="ExternalInput")
    else:
        x_d = nc.dram_tensor("x", [NT, C], f32r, kind="ExternalInput")
    wqk_d = nc.dram_tensor("wqk", [C, 2 * C], f32r, kind="ExternalInput")
    wv_d = nc.dram_tensor("wv", [C, C], f32r, kind="ExternalInput")
    wp_d = nc.dram_tensor("wp", [C, C], f32r, kind="ExternalInput")
    bqk_d = nc.dram_tensor("bqk", [2 * C], f32, kind="ExternalInput")
    beff_d = nc.dram_tensor("beff", [1, C], f32, kind="ExternalInput")
    mexp_d = nc.dram_tensor("mexp", [H, TG, TG], bf16, kind="ExternalInput")
    y_d = nc.dram_tensor("y", [NT, C], f32, kind="ExternalOutput")

    with tile.TileContext(nc) as tc:
        with (
            tc.tile_pool(name="stat", bufs=1) as stat,
            tc.tile_pool(name="xp", bufs=cfg["xp"]) as xp,
            tc.tile_pool(name="qkp", bufs=cfg["qkp"]) as qkp,
            tc.tile_pool(name="attp", bufs=cfg["attp"]) as attp,
            tc.tile_pool(name="etsp", bufs=cfg["etsp"]) as etsp,
            tc.tile_pool(name="io", bufs=cfg["io"]) as io,
            tc.tile_pool(name="ps_vq", bufs=cfg["vq"], space="PSUM") as ps_vq,
            tc.tile_pool(name="ps_s", bufs=cfg["s"], space="PSUM") as ps_s,
            tc.tile_pool(name="ps_et", bufs=cfg["et"], space="PSUM") as ps_et,
            tc.tile_pool(name="ps_av", bufs=cfg["av"], space="PSUM") as ps_av,
        ):
            mlist = MACROS if macros is None else macros

            def macro_params(b0, gsizes):
                tgs = [g * N for g in gsizes]
                goffs = np.concatenate([[0], np.cumsum(tgs)]).tolist()
                Tm = goffs[-1]
                t0 = b0 * N
                # transposed-DMA row count: multiple of 16 (476->480; 272 ok)
                rows = Tm if Tm % 16 == 0 else min(-(-Tm // 16) * 16, NT - t0)
                return tgs, goffs, Tm, t0, rows

            def load_x(t0, rows, tgs, goffs):
                if cfg["dmat"]:
                    xhi_t = xp.tile([128, 4, 480], bf16, tag="xhi")
                    xlo_t = xp.tile([128, 4, 480], bf16, tag="xlo")
                    for c in range(4):
                        nc.sync.dma_start_transpose(
                            out=xhi_t[:, c, :rows],
                            in_=xhi_d[t0:t0 + rows, c * 128:(c + 1) * 128])
                        nc.sync.dma_start_transpose(
                            out=xlo_t[:, c, :rows],
                            in_=xlo_d[t0:t0 + rows, c * 128:(c + 1) * 128])
                    return xhi_t, xlo_t
                xgs = []
                for gi in range(len(tgs)):
                    tg, go = tgs[gi], goffs[gi]
                    xg = io.tile([TG, C], f32, tag=f"xg{gi}")
                    nc.sync.dma_start(out=xg[:tg, :],
                                      in_=x_d[t0 + go:t0 + go + tg, :])
                    xgs.append(xg)
                return xgs

            # macro 0's x DMAs first so they are at the head of the DMA queue
            _p0 = macro_params(*mlist[0])
            xpair0 = load_x(_p0[3], _p0[4], _p0[0], _p0[1])

            # ---- static weights: consolidated SWDGE loads (gpsimd queue,
            # no HWDGE contention with the x/y stream), ordered by first use
            wv_all = stat.tile([128, 4, C], f32r, tag="wv")
            nc.gpsimd.dma_start(out=wv_all,
                                in_=wv_d.rearrange("(c p) n -> p c n", c=4))
            wv_sb = [wv_all[:, c, :] for c in range(4)]
            wqk_all = stat.tile([128, 4, 2 * C], f32r, tag="wqk")
            nc.gpsimd.dma_start(out=wqk_all,
                                in_=wqk_d.rearrange("(c p) n -> p c n", c=4))
            wqk_sb = [wqk_all[:, c, :] for c in range(4)]
            bqk_sb = stat.tile([128, 8], f32, tag="bqk")
            nc.gpsimd.dma_start(out=bqk_sb,
                                in_=bqk_d.rearrange("(m p) -> p m", m=8))
            mexp_all = stat.tile([TG, H, TG], bf16, tag="mexp")
            nc.gpsimd.dma_start(out=mexp_all,
                                in_=mexp_d.rearrange("h p n -> p h n"))
            mexp_sb = [mexp_all[:, h, :] for h in range(H)]
            idh = stat.tile([128, 128], bf16, tag="idh")
            make_identity(nc, idh)
            if not cfg["dmat"]:
                idf = stat.tile([128, 128], f32, tag="idf")
                make_identity(nc, idf)
            wp_all = stat.tile([128, 4, C], f32r, tag="wp")
            nc.gpsimd.dma_start(out=wp_all,
                                in_=wp_d.rearrange("(c p) n -> p c n", c=4))
            wp_sb = [wp_all[:, c, :] for c in range(4)]
            beff_sb = stat.tile([128, C], f32, tag="beff")
            nc.gpsimd.dma_start(out=beff_sb,
                                in_=beff_d[0:1, :].partition_broadcast(128))

            for mi, (b0, gsizes) in enumerate(mlist):
                ng = len(gsizes)
                tgs, goffs, Tm, t0, rows = macro_params(b0, gsizes)
                if mi == 0:
                    xpair = xpair0
                else:
                    xpair = load_x(t0, rows, tgs, goffs)
                xT = xp.tile([128, 4, 480], f32r, tag="xT")
                if cfg["dmat"]:
                    xhi_t, xlo_t = xpair
                    # gpsimd adds in f32 (its f32r rounding is broken on HW),
                    # then a gpsimd cast DMA relabels to f32r for the matmuls
                    # (raw f32 bits as f32r -- same as the weight loads).
                    if cfg["xadd"] == "dve":
                        with tc.high_priority(offset=cfg["xprio"] or None):
                            for c in range(4):
                                nc.vector.tensor_tensor(out=xT[:, c, :Tm],
                                                        in0=xhi_t[:, c, :Tm],
                                                        in1=xlo_t[:, c, :Tm],
                                                        op=AluOpType.add)
                    else:
                        x32 = xp.tile([128, 4, 480], f32, tag="x32")
                        eng = nc.vector if mi < 2 else nc.gpsimd
                        for c in range(4):
                            eng.tensor_tensor(out=x32[:, c, :Tm],
                                              in0=xhi_t[:, c, :Tm],
                                              in1=xlo_t[:, c, :Tm],
                                              op=AluOpType.add)
                        nc.gpsimd.dma_start(out=xT[:, :, :Tm],
                                            in_=x32[:, :, :Tm])
                else:
                    for gi in range(ng):
                        tg, go = tgs[gi], goffs[gi]
                        tps = ps_et.tile([128, 4, 128], f32, tag="xt")
                        for c in range(4):
                            nc.tensor.matmul(tps[:, c, :tg],
                                             xpair[gi][:tg, c * 128:(c + 1) * 128],
                                             idf[:tg, :tg], is_transpose=True,
                                             start=True, stop=True)
                        nc.vector.tensor_copy(out=xT[:, :, go:go + tg],
                                              in_=tps[:, :, :tg])

                # ---- v projection per group ----
                v_tiles = []
                for gi in range(ng):
                    tg, go = tgs[gi], goffs[gi]
                    vps = ps_vq.tile([TG, C], f32, tag="vq")
                    for c in range(4):
                        nc.tensor.matmul(vps[:tg, :], xT[:, c, go:go + tg],
                                         wv_sb[c], start=(c == 0), stop=(c == 3))
                    vg = io.tile([TG, C], bf16, tag=f"vg{gi}")
                    if cfg["vg_dve"]:
                        nc.vector.tensor_copy(out=vg[:tg, :], in_=vps[:tg, :])
                    else:
                        nc.scalar.activation(out=vg[:tg, :], in_=vps[:tg, :],
                                             func=Act.Copy)
                    v_tiles.append(vg)

                # ---- qk projection: qkT chunks [128, Tm] (rows of [q;k]) ----
                qkT = []
                for m in range(8):
                    qps = ps_vq.tile([128, 512], f32, tag="vq")
                    for c in range(4):
                        nc.tensor.matmul(qps[:, :Tm],
                                         wqk_sb[c][:, m * 128:(m + 1) * 128],
                                         xT[:, c, :Tm],
                                         start=(c == 0), stop=(c == 3))
                    qk = qkp.tile([128, Tm], bf16, tag=f"qkT{m}")
                    if m < cfg["qk_dve"]:
                        nc.vector.tensor_scalar_add(qk[:, :], qps[:, :Tm],
                                                    bqk_sb[:, m:m + 1])
                    else:
                        nc.scalar.activation(out=qk[:, :], in_=qps[:, :Tm],
                                             func=Act.Identity,
                                             bias=bqk_sb[:, m:m + 1])
                    qkT.append(qk)

                # ---- attention, head pairs ----
                z = attp.tile([TG, 32], f32, tag="z")
                if gsizes != [7] * GPM:
                    nc.vector.memset(z, 1.0)
                rzb = attp.tile([TG, 32], f32, tag="rzb")
                for p in range(4):
                    avps = ps_av.tile([128, 512], f32, tag="av")
                    for h in (2 * p, 2 * p + 1):
                        ro = (h % 2) * 64
                        sps = ps_s.tile([TG, 4, 128], f32, tag="s")
                        for gi in range(ng):
                            tg, go = tgs[gi], goffs[gi]
                            nc.tensor.matmul(sps[:tg, gi, :tg],
                                             qkT[p][ro:ro + 64, go:go + tg],
                                             qkT[4 + p][ro:ro + 64, go:go + tg],
                                             start=True, stop=True)
                        er = attp.tile([TG, ng, TG], bf16, tag="er")
                        if gsizes == [7] * GPM:
                            nc.scalar.activation(out=er[:, :, :],
                                                 in_=sps[:, :, :TG],
                                                 func=Act.Exp)
                        else:
                            for gi in range(ng):
                                tg = tgs[gi]
                                nc.scalar.activation(
                                    out=er[:tg, gi, :tg],
                                    in_=sps[:tg, gi, :tg], func=Act.Exp)
                        em = attp.tile([TG, ng, TG], bf16, tag="em")
                        for gi in range(ng):
                            tg = tgs[gi]
                            if cfg["ttr"] == 1:
                                nc.vector.tensor_tensor_reduce(
                                    out=em[:tg, gi, :tg], in0=er[:tg, gi, :tg],
                                    in1=mexp_sb[h][:tg, :tg], scale=1.0,
                                    scalar=0.0, op0=AluOpType.mult,
                                    op1=AluOpType.add,
                                    accum_out=z[:tg, h * 4 + gi: h * 4 + gi + 1])
                            elif cfg["ttr"] == 2:
                                nc.vector.tensor_tensor(
                                    out=em[:tg, gi, :tg], in0=er[:tg, gi, :tg],
                                    in1=mexp_sb[h][:tg, :tg],
                                    op=AluOpType.mult)
                                nc.vector.tensor_reduce(
                                    out=z[:tg, h * 4 + gi: h * 4 + gi + 1],
                                    in_=em[:tg, gi, :tg],
                                    axis=mybir.AxisListType.X,
                                    op=AluOpType.add)
                            else:
                                nc.vector.scalar_tensor_tensor(
                                    out=em[:tg, gi, :tg],
                                    in0=er[:tg, gi, :tg], scalar=1.0,
                                    in1=mexp_sb[h][:tg, :tg],
                                    op0=AluOpType.mult, op1=AluOpType.mult,
                                    accum_out=z[:tg, h * 4 + gi: h * 4 + gi + 1])
                        nc.vector.reciprocal(out=rzb[:, h * 4:h * 4 + ng],
                                             in_=z[:, h * 4:h * 4 + ng])
                        en = attp.tile([TG, ng, TG], bf16, tag="en")
                        for gi in range(ng):
                            tg = tgs[gi]
                            nc.vector.tensor_scalar_mul(
                                en[:tg, gi, :tg], em[:tg, gi, :tg],
                                rzb[:tg, h * 4 + gi: h * 4 + gi + 1])
                        etps = ps_et.tile([128, 4, 256], bf16, tag="et")
                        for gi in range(ng):
                            tg = tgs[gi]
                            nc.tensor.matmul(etps[:tg, gi, :tg],
                                             en[:tg, gi, :tg], idh[:tg, :tg],
                                             is_transpose=True, start=True,
                                             stop=True)
                        ets = etsp.tile([128, ng, TG + 1], bf16, tag="ets")
                        etseng = nc.scalar if cfg["ets_act"] else nc.vector
                        if gsizes == [7] * GPM:
                            if cfg["ets_act"]:
                                nc.scalar.activation(out=ets[:TG, :, :TG],
                                                     in_=etps[:TG, :GPM, :TG],
                                                     func=Act.Copy)
                            else:
                                nc.vector.tensor_copy(out=ets[:TG, :, :TG],
                                                      in_=etps[:TG, :GPM, :TG])
                        else:
                            for gi in range(ng):
                                tg = tgs[gi]
                                if cfg["ets_act"]:
                                    nc.scalar.activation(
                                        out=ets[:tg, gi, :tg],
                                        in_=etps[:tg, gi, :tg], func=Act.Copy)
                                else:
                                    nc.vector.tensor_copy(
                                        out=ets[:tg, gi, :tg],
                                        in_=etps[:tg, gi, :tg])
                        for gi in range(ng):
                            tg, go = tgs[gi], goffs[gi]
                            nc.tensor.matmul(
                                avps[ro:ro + 64, go:go + tg],
                                v_tiles[gi][:tg, h * 64:(h + 1) * 64],
                                ets[:tg, gi, :tg],
                                tile_position=(0, ro) if cfg["tilepos"] else None,
                                start=True, stop=True)
                    attT = qkp.tile([128, Tm], f32r, tag=f"attT{p}")
                    # DVE: must produce true f32r for the proj matmul
                    nc.vector.tensor_copy(out=attT[:, :], in_=avps[:, :Tm])
                    qkT.append(attT)  # keep alive; index 8+p

                # ---- output projection ----
                for gi in range(ng):
                    tg, go = tgs[gi], goffs[gi]
                    fps = ps_av.tile([TG, C], f32, tag="av")
                    for c in range(4):
                        nc.tensor.matmul(fps[:tg, :], qkT[8 + c][:, go:go + tg],
                                         wp_sb[c], start=(c == 0), stop=(c == 3))
                    yg = io.tile([TG, C], f32, tag="yg")
                    if cfg["ygp"]:
                        yr = io.tile([TG, C], f32, tag="yr")
                        nc.scalar.activation(out=yr[:tg, :], in_=fps[:tg, :],
                                             func=Act.Copy)
                        nc.gpsimd.tensor_tensor(out=yg[:tg, :],
                                                in0=yr[:tg, :],
                                                in1=beff_sb[:tg, :],
                                                op=AluOpType.add)
                    else:
                        nc.vector.tensor_tensor(out=yg[:tg, :],
                                                in0=fps[:tg, :],
                                                in1=beff_sb[:tg, :],
                                                op=AluOpType.add)
                    nc.sync.dma_start(out=y_d[t0 + go: t0 + go + tg, :],
                                      in_=yg[:tg, :])

    nc.compile()
    return nc


def _host_prep(x, W_qkv, b_qkv, outer, alpha, W_proj, b_proj):
    import ml_dtypes

    bf16 = ml_dtypes.bfloat16
    scale = np.float32(HD ** -0.5)
    x = np.ascontiguousarray(np.asarray(x, dtype=np.float32))
    W_qkv = np.asarray(W_qkv, dtype=np.float32)
    b_qkv = np.asarray(b_qkv, dtype=np.float32)
    outer = np.asarray(outer, dtype=np.float32)
    alpha = np.asarray(alpha, dtype=np.float32)
    W_proj = np.asarray(W_proj, dtype=np.float32)
    b_proj = np.asarray(b_proj, dtype=np.float32)

    xhi = x.astype(bf16)
    xlo = (x - xhi.astype(np.float32)).astype(bf16)

    wqk = np.concatenate([W_qkv[:, :C] * scale, W_qkv[:, C:2 * C]], axis=1)
    bqk = np.concatenate([b_qkv[:C] * scale, b_qkv[C:2 * C]])
    wv = W_qkv[:, 2 * C:]
    bv = b_qkv[2 * C:]
    beff = (b_proj + bv @ W_proj)[None, :]

    # multiplicative mask: 0 off-block, exp(alpha*outer*scale) on diag blocks
    base = np.zeros((H, TG, TG), dtype=np.float32)
    bias = np.exp((alpha[0] * scale) * outer)  # [H, 17, 17]
    for i in range(G):
        base[:, i * N:(i + 1) * N, i * N:(i + 1) * N] = bias
    mexp = base.astype(bf16)

    shared = {
        "wqk": np.ascontiguousarray(wqk),
        "wv": np.ascontiguousarray(wv),
        "wp": np.ascontiguousarray(W_proj),
        "bqk": np.ascontiguousarray(bqk),
        "beff": np.ascontiguousarray(beff),
        "mexp": np.ascontiguousarray(mexp),
    }
    return xhi, xlo, shared


def kernel(x, W_qkv, b_qkv, outer, alpha, W_proj, b_proj, _trace=False):
    from concourse.bass_utils import run_bass_kernel_spmd

    if "nc" not in _CACHE:
        _CACHE["nc"] = _build_program(cfg=CFG)
    nc = _CACHE["nc"]

    xhi, xlo, shared = _host_prep(x, W_qkv, b_qkv, outer, alpha, W_proj, b_proj)
    use_dmat = CFG.get("dmat", 1)
    in_maps = []
    for c in range(NCORES):
        m = dict(shared)
        if use_dmat:
            m["xhi"] = np.ascontiguousarray(
                xhi[c * BC:(c + 1) * BC].reshape(NT, C))
            m["xlo"] = np.ascontiguousarray(
                xlo[c * BC:(c + 1) * BC].reshape(NT, C))
        else:
            m["x"] = np.ascontiguousarray(
                np.asarray(x, dtype=np.float32)[c * BC:(c + 1) * BC]
                .reshape(NT, C))
        in_maps.append(m)

    res = run_bass_kernel_spmd(nc, in_maps, core_ids=list(range(NCORES)),
                               trace=_trace)
    out = np.concatenate(
        [res.results[c]["y"].reshape(BC, N, C) for c in range(NCORES)], axis=0)
    if _trace:
        _CACHE["last_result"] = res
    return out


# revision 54
# speedup vs baseline: 1.0709x; 1.0181x over previous
"""Trainium2 Bass kernel for nn_Attention: batched small-N attention.

Reference computation (per batch b of 8192, tokens N=17, C=512, H=8 heads, HD=64):
    qkv = x @ W_qkv + b_qkv
    q,k,v split/reshaped; logits = (q @ k^T + alpha*outer)*scale; A = softmax
    out = (A @ v reshaped back) @ W_proj + b_proj

Strategy: pure data parallel over B across 8 cores (1024 batches/core).
Per core, batches are packed into groups of 7 (119 tokens <= 128 partitions) and
macro-tiles of 4 groups (476 tokens) so every big matmul runs with free dim >=
256 in float32r (1 cyc/row); attention-internal matmuls run in bf16.

Engine choreography (the v1 kernel was DVE-bound at 90%; this one holds PE at
~90% with DVE/ACT at ~80/70%):
  - x is shipped as a bf16 hi/lo pair and loaded TRANSPOSED by the DMA xbar
    (dma_start_transpose, 2-byte dtypes only); the DVE re-adds hi+lo to f32r.
    The tensor engine never runs x transposes.
  - qk / v PSUM evacuations ride the Scalar engine (activation Copy/Identity,
    with the folded q/k bias as the per-partition Identity bias).
  - Cross-batch attention inside a group is killed with a MULTIPLICATIVE mask
    exp(mask) (0 off-block, exp(alpha*outer*scale) on-block) fused into one
    scalar_tensor_tensor per (head, group) whose accum_out also yields the
    softmax denominators z (per-query-partition, so 1/z is a cheap
    per-partition tensor_scalar).  exp itself reads S straight out of PSUM on
    the Scalar engine with no bias add.
  - A^T transposes run on PE (bf16 identity matmuls); A^T@V runs as col-tiled
    (tile_position) head pairs writing both heads into one PSUM tile that IS
    the attT layout chunk for the output projection.
  - output bias add runs on GpSimd after an ACT evacuation.
HW-validated quirks: tensor_tensor_reduce crashes the device (use
scalar_tensor_tensor+accum_out); gpsimd mis-rounds float32r outputs and the
BIR verifier requires f32r matmul operands to be produced as f32r, so every
f32r elementwise producer is the DVE; PSUM tiles must be 2KB bank-aligned.

All biases are folded host-side:
  - scale into W_q/b_q
  - b_qkv (q/k parts) added during the qk PSUM->SBUF evacuation
  - b_v and b_proj into one effective output bias: beff = b_proj + b_v @ W_proj
  - alpha*outer*scale into the multiplicative mask
Softmax needs no max-subtraction: |logits| <= ~1 by construction of the inputs.
"""

import numpy as np

B, N, C, H, HD = 8192, 17, 512, 8, 64
NCORES = 8
BC = B // NCORES            # batches per core
NT = BC * N                 # tokens per core
G = 7                       # batches per group
TG = G * N                  # 119 tokens per group
GPM = 4                     # groups per (full) macro tile

# 1024 = 36 * 28 + 16;  final macro = groups of (7, 7, 2) batches
MACROS = [(m * (G * GPM), [7, 7, 7, 7]) for m in range(36)] + [(1008, [7, 7, 2])]

_CACHE = {}

# runtime-selected implementation details (validated on HW by probes)
CFG = {}


def _build_program(macros=None, cfg=None):
    cfg = dict(cfg or {})
    for k, v in dict(xp=2, qkp=2, attp=4, etsp=4, io=8,
                     vq=4, s=1, et=2, av=1,
                     qk_dve=0, attT_dve=1, vg_dve=0, xadd="dve", ygp=1,
                     ets_act=0, xprio=0, yt=1,
                     dmat=1, tilepos=1, ttr=0, gpadd=1).items():
        cfg.setdefault(k, v)
    import concourse.bass as bass
    import concourse.mybir as mybir
    import concourse.tile as tile
    from concourse import bacc
    from concourse.alu_op_type import AluOpType
    from concourse.masks import make_identity

    f32 = mybir.dt.float32
    f32r = mybir.dt.float32r
    bf16 = mybir.dt.bfloat16
    Act = mybir.ActivationFunctionType

    nc = bacc.Bacc("TRN2", target_bir_lowering=False, debug=False,
                   num_devices=NCORES)

    if cfg["dmat"]:
        xhi_d = nc.dram_tensor("xhi", [NT, C], bf16, kind="ExternalInput")
        xlo_d = nc.dram_tensor("xlo", [NT, C], bf16, kind="ExternalInput")
    else:
        x_d = nc.dram_tensor("x", [NT, C], f32r, kind="ExternalInput")
    if cfg["wlo"]:
        wqk_hd = nc.dram_tensor("wqkh", [C, 2 * C], bf16, kind="ExternalInput")
        wqk_ld = nc.dram_tensor("wqkl", [C, 2 * C], bf16, kind="ExternalInput")
        wv_hd = nc.dram_tensor("wvh", [C, C], bf16, kind="ExternalInput")
        wv_ld = nc.dram_tensor("wvl", [C, C], bf16, kind="ExternalInput")
        wp_hd = nc.dram_tensor("wph", [C, C], bf16, kind="ExternalInput")
        wp_ld = nc.dram_tensor("wpl", [C, C], bf16, kind="ExternalInput")
    else:
        wqk_d = nc.dram_tensor("wqk", [C, 2 * C], f32r, kind="ExternalInput")
        wv_d = nc.dram_tensor("wv", [C, C], f32r, kind="ExternalInput")
        wp_d = nc.dram_tensor("wp", [C, C], f32r, kind="ExternalInput")
    bqk_d = nc.dram_tensor("bqk", [2 * C], f32, kind="ExternalInput")
    beff_d = nc.dram_tensor("beff", [1, C], f32, kind="ExternalInput")
    mexp_d = nc.dram_tensor("mexp", [H, TG, TG], bf16, kind="ExternalInput")
    if cfg["yt"]:
        y_d = nc.dram_tensor("y", [C, NT], f32, kind="ExternalOutput")
    else:
        y_d = nc.dram_tensor("y", [NT, C], f32, kind="ExternalOutput")

    with tile.TileContext(nc) as tc:
        with (
            tc.tile_pool(name="stat", bufs=1) as stat,
            tc.tile_pool(name="xp", bufs=cfg["xp"]) as xp,
            tc.tile_pool(name="qkp", bufs=cfg["qkp"]) as qkp,
            tc.tile_pool(name="attp", bufs=cfg["attp"]) as attp,
            tc.tile_pool(name="etsp", bufs=cfg["etsp"]) as etsp,
            tc.tile_pool(name="io", bufs=cfg["io"]) as io,
            tc.tile_pool(name="ps_vq", bufs=cfg["vq"], space="PSUM") as ps_vq,
            tc.tile_pool(name="ps_s", bufs=cfg["s"], space="PSUM") as ps_s,
            tc.tile_pool(name="ps_et", bufs=cfg["et"], space="PSUM") as ps_et,
            tc.tile_pool(name="ps_av", bufs=cfg["av"], space="PSUM") as ps_av,
        ):
            mlist = MACROS if macros is None else macros

            def macro_params(b0, gsizes):
                tgs = [g * N for g in gsizes]
                goffs = np.concatenate([[0], np.cumsum(tgs)]).tolist()
                Tm = goffs[-1]
                t0 = b0 * N
                # transposed-DMA row count: multiple of 16 (476->480; 272 ok)
                rows = Tm if Tm % 16 == 0 else min(-(-Tm // 16) * 16, NT - t0)
                return tgs, goffs, Tm, t0, rows

            def load_x(t0, rows, tgs, goffs):
                if cfg["dmat"]:
                    xhi_t = xp.tile([128, 4, 480], bf16, tag="xhi")
                    xlo_t = xp.tile([128, 4, 480], bf16, tag="xlo")
                    for c in range(4):
                        nc.sync.dma_start_transpose(
                            out=xhi_t[:, c, :rows],
                            in_=xhi_d[t0:t0 + rows, c * 128:(c + 1) * 128])
                        nc.sync.dma_start_transpose(
                            out=xlo_t[:, c, :rows],
                            in_=xlo_d[t0:t0 + rows, c * 128:(c + 1) * 128])
                    return xhi_t, xlo_t
                xgs = []
                for gi in range(len(tgs)):
                    tg, go = tgs[gi], goffs[gi]
                    xg = io.tile([TG, C], f32, tag=f"xg{gi}")
                    nc.sync.dma_start(out=xg[:tg, :],
                                      in_=x_d[t0 + go:t0 + go + tg, :])
                    xgs.append(xg)
                return xgs

            # macro 0's x DMAs first so they are at the head of the DMA queue
            _p0 = macro_params(*mlist[0])
            xpair0 = load_x(_p0[3], _p0[4], _p0[0], _p0[1])

            # ---- static weights: consolidated SWDGE loads (gpsimd queue,
            # no HWDGE contention with the x/y stream), ordered by first use
            def load_w(dst, f32r_src, hd, ld, nsub):
                if not cfg["wlo"]:
                    if cfg["wsplit"]:
                        for c in range(4):
                            nc.gpsimd.dma_start(
                                out=dst[:, c, :],
                                in_=f32r_src[c * 128:(c + 1) * 128, :])
                    else:
                        nc.gpsimd.dma_start(
                            out=dst,
                            in_=f32r_src.rearrange("(c p) n -> p c n", c=4))
                    return
                wh = stat.tile(list(dst.shape), bf16, tag=f"h{dst.name}")
                wl = stat.tile(list(dst.shape), bf16, tag=f"l{dst.name}")
                nc.gpsimd.dma_start(
                    out=wh, in_=hd.rearrange("(c p) n -> p c n", c=4))
                nc.gpsimd.dma_start(
                    out=wl, in_=ld.rearrange("(c p) n -> p c n", c=4))
                # split adds so downstream chunks unblock progressively
                for c in range(4):
                    nc.vector.tensor_tensor(out=dst[:, c, :], in0=wh[:, c, :],
                                            in1=wl[:, c, :], op=AluOpType.add)

            wv_all = stat.tile([128, 4, C], f32r, tag="wv")
            load_w(wv_all, wv_d if not cfg["wlo"] else None,
                   wv_hd if cfg["wlo"] else None,
                   wv_ld if cfg["wlo"] else None, 4)
            wv_sb = [wv_all[:, c, :] for c in range(4)]
            wqk_all = stat.tile([128, 4, 2 * C], f32r, tag="wqk")
            load_w(wqk_all, wqk_d if not cfg["wlo"] else None,
                   wqk_hd if cfg["wlo"] else None,
                   wqk_ld if cfg["wlo"] else None, 4)
            wqk_sb = [wqk_all[:, c, :] for c in range(4)]
            bqk_sb = stat.tile([128, 8], f32, tag="bqk")
            nc.gpsimd.dma_start(out=bqk_sb,
                                in_=bqk_d.rearrange("(m p) -> p m", m=8))
            mexp_all = stat.tile([TG, H, TG], bf16, tag="mexp")
            nc.gpsimd.dma_start(out=mexp_all,
                                in_=mexp_d.rearrange("h p n -> p h n"))
            mexp_sb = [mexp_all[:, h, :] for h in range(H)]
            idh = stat.tile([128, 128], bf16, tag="idh")
            make_identity(nc, idh)
            if not cfg["dmat"]:
                idf = stat.tile([128, 128], f32, tag="idf")
                make_identity(nc, idf)
            wp_all = stat.tile([128, 4, C], f32r, tag="wp")
            load_w(wp_all, wp_d if not cfg["wlo"] else None,
                   wp_hd if cfg["wlo"] else None,
                   wp_ld if cfg["wlo"] else None, 4)
            wp_sb = [wp_all[:, c, :] for c in range(4)]
            if cfg["yt"]:
                beff_sb = stat.tile([128, 4], f32, tag="beff")
                nc.gpsimd.dma_start(
                    out=beff_sb,
                    in_=beff_d[0:1, :].rearrange("a (m p) -> p (a m)", m=4))
            else:
                beff_sb = stat.tile([128, C], f32, tag="beff")
                nc.gpsimd.dma_start(out=beff_sb,
                                    in_=beff_d[0:1, :].partition_broadcast(128))

            for mi, (b0, gsizes) in enumerate(mlist):
                ng = len(gsizes)
                tgs, goffs, Tm, t0, rows = macro_params(b0, gsizes)
                if mi == 0:
                    xpair = xpair0
                else:
                    xpair = load_x(t0, rows, tgs, goffs)
                xT = xp.tile([128, 4, 480], f32r, tag="xT")
                if cfg["dmat"]:
                    xhi_t, xlo_t = xpair
                    # gpsimd adds in f32 (its f32r rounding is broken on HW),
                    # then a gpsimd cast DMA relabels to f32r for the matmuls
                    # (raw f32 bits as f32r -- same as the weight loads).
                    if cfg["xadd"] == "dve":
                        with tc.high_priority(offset=cfg["xprio"] or None):
                            for c in range(4):
                                nc.vector.tensor_tensor(out=xT[:, c, :Tm],
                                                        in0=xhi_t[:, c, :Tm],
                                                        in1=xlo_t[:, c, :Tm],
                                                        op=AluOpType.add)
                    else:
                        x32 = xp.tile([128, 4, 480], f32, tag="x32")
                        eng = nc.vector if mi < 2 else nc.gpsimd
                        for c in range(4):
                            eng.tensor_tensor(out=x32[:, c, :Tm],
                                              in0=xhi_t[:, c, :Tm],
                                              in1=xlo_t[:, c, :Tm],
                                              op=AluOpType.add)
                        nc.gpsimd.dma_start(out=xT[:, :, :Tm],
                                            in_=x32[:, :, :Tm])
                else:
                    for gi in range(ng):
                        tg, go = tgs[gi], goffs[gi]
                        tps = ps_et.tile([128, 4, 128], f32, tag="xt")
                        for c in range(4):
                            nc.tensor.matmul(tps[:, c, :tg],
                                             xpair[gi][:tg, c * 128:(c + 1) * 128],
                                             idf[:tg, :tg], is_transpose=True,
                                             start=True, stop=True)
                        nc.vector.tensor_copy(out=xT[:, :, go:go + tg],
                                              in_=tps[:, :, :tg])

                # ---- v projection per group ----
                v_tiles = []
                for gi in range(ng):
                    tg, go = tgs[gi], goffs[gi]
                    vps = ps_vq.tile([TG, C], f32, tag="vq")
                    for c in range(4):
                        nc.tensor.matmul(vps[:tg, :], xT[:, c, go:go + tg],
                                         wv_sb[c], start=(c == 0), stop=(c == 3))
                    vg = io.tile([TG, C], bf16, tag=f"vg{gi}")
                    if cfg["vg_dve"]:
                        nc.vector.tensor_copy(out=vg[:tg, :], in_=vps[:tg, :])
                    else:
                        nc.scalar.activation(out=vg[:tg, :], in_=vps[:tg, :],
                                             func=Act.Copy)
                    v_tiles.append(vg)

                # ---- qk projection: qkT chunks [128, Tm] (rows of [q;k]) ----
                qkT = []
                for m in range(8):
                    qps = ps_vq.tile([128, 512], f32, tag="vq")
                    for c in range(4):
                        nc.tensor.matmul(qps[:, :Tm],
                                         wqk_sb[c][:, m * 128:(m + 1) * 128],
                                         xT[:, c, :Tm],
                                         start=(c == 0), stop=(c == 3))
                    qk = qkp.tile([128, Tm], bf16, tag=f"qkT{m}")
                    if m < cfg["qk_dve"]:
                        nc.vector.tensor_scalar_add(qk[:, :], qps[:, :Tm],
                                                    bqk_sb[:, m:m + 1])
                    else:
                        nc.scalar.activation(out=qk[:, :], in_=qps[:, :Tm],
                                             func=Act.Identity,
                                             bias=bqk_sb[:, m:m + 1])
                    qkT.append(qk)

                # ---- attention, head pairs ----
                z = attp.tile([TG, 32], f32, tag="z")
                if gsizes != [7] * GPM:
                    nc.vector.memset(z, 1.0)
                rzb = attp.tile([TG, 32], f32, tag="rzb")
                for p in range(4):
                    avps = ps_av.tile([128, 512], f32, tag="av")
                    for h in (2 * p, 2 * p + 1):
                        ro = (h % 2) * 64
                        sps = ps_s.tile([TG, 4, 128], f32, tag="s")
                        for gi in range(ng):
                            tg, go = tgs[gi], goffs[gi]
                            nc.tensor.matmul(sps[:tg, gi, :tg],
                                             qkT[p][ro:ro + 64, go:go + tg],
                                             qkT[4 + p][ro:ro + 64, go:go + tg],
                                             start=True, stop=True)
                        er = attp.tile([TG, ng, TG], bf16, tag="er")
                        if gsizes == [7] * GPM:
                            nc.scalar.activation(out=er[:, :, :],
                                                 in_=sps[:, :, :TG],
                                                 func=Act.Exp)
                        else:
                            for gi in range(ng):
                                tg = tgs[gi]
                                nc.scalar.activation(
                                    out=er[:tg, gi, :tg],
                                    in_=sps[:tg, gi, :tg], func=Act.Exp)
                        em = attp.tile([TG, ng, TG], bf16, tag="em")
                        for gi in range(ng):
                            tg = tgs[gi]
                            if cfg["ttr"] == 1:
                                nc.vector.tensor_tensor_reduce(
                                    out=em[:tg, gi, :tg], in0=er[:tg, gi, :tg],
                                    in1=mexp_sb[h][:tg, :tg], scale=1.0,
                                    scalar=0.0, op0=AluOpType.mult,
                                    op1=AluOpType.add,
                                    accum_out=z[:tg, h * 4 + gi: h * 4 + gi + 1])
                            elif cfg["ttr"] == 2:
                                nc.vector.tensor_tensor(
                                    out=em[:tg, gi, :tg], in0=er[:tg, gi, :tg],
                                    in1=mexp_sb[h][:tg, :tg],
                                    op=AluOpType.mult)
                                nc.vector.tensor_reduce(
                                    out=z[:tg, h * 4 + gi: h * 4 + gi + 1],
                                    in_=em[:tg, gi, :tg],
                                    axis=mybir.AxisListType.X,
                                    op=AluOpType.add)
                            else:
                                nc.vector.scalar_tensor_tensor(
                                    out=em[:tg, gi, :tg],
                                    in0=er[:tg, gi, :tg], scalar=1.0,
                                    in1=mexp_sb[h][:tg, :tg],
                                    op0=AluOpType.mult, op1=AluOpType.mult,
                                    accum_out=z[:tg, h * 4 + gi: h * 4 + gi + 1])
                        nc.vector.reciprocal(out=rzb[:, h * 4:h * 4 + ng],
                                             in_=z[:, h * 4:h * 4 + ng])
                        en = attp.tile([TG, ng, TG], bf16, tag="en")
                        for gi in range(ng):
                            tg = tgs[gi]
                            nc.vector.tensor_scalar_mul(
                                en[:tg, gi, :tg], em[:tg, gi, :tg],
                                rzb[:tg, h * 4 + gi: h * 4 + gi + 1])
                        etps = ps_et.tile([128, 4, 256], bf16, tag="et")
                        for gi in range(ng):
                            tg = tgs[gi]
                            nc.tensor.matmul(etps[:tg, gi, :tg],
                                             en[:tg, gi, :tg], idh[:tg, :tg],
                                             is_transpose=True, start=True,
                                             stop=True)
                        ets = etsp.tile([128, ng, TG + 1], bf16, tag="ets")
                        etseng = nc.scalar if cfg["ets_act"] else nc.vector
                        if gsizes == [7] * GPM:
                            if cfg["ets_act"]:
                                nc.scalar.activation(out=ets[:TG, :, :TG],
                                                     in_=etps[:TG, :GPM, :TG],
                                                     func=Act.Copy)
                            else:
                                nc.vector.tensor_copy(out=ets[:TG, :, :TG],
                                                      in_=etps[:TG, :GPM, :TG])
                        else:
                            for gi in range(ng):
                                tg = tgs[gi]
                                if cfg["ets_act"]:
                                    nc.scalar.activation(
                                        out=ets[:tg, gi, :tg],
                                        in_=etps[:tg, gi, :tg], func=Act.Copy)
                                else:
                                    nc.vector.tensor_copy(
                                        out=ets[:tg, gi, :tg],
                                        in_=etps[:tg, gi, :tg])
                        for gi in range(ng):
                            tg, go = tgs[gi], goffs[gi]
                            nc.tensor.matmul(
                                avps[ro:ro + 64, go:go + tg],
                                v_tiles[gi][:tg, h * 64:(h + 1) * 64],
                                ets[:tg, gi, :tg],
                                tile_position=(0, ro) if cfg["tilepos"] else None,
                                start=True, stop=True)
                    attT = qkp.tile([128, Tm], f32r, tag=f"attT{p}")
                    # DVE: must produce true f32r for the proj matmul
                    nc.vector.tensor_copy(out=attT[:, :], in_=avps[:, :Tm])
                    qkT.append(attT)  # keep alive; index 8+p

                # ---- output projection ----
                if cfg["yt"]:
                    # weight-stationary, channel-major y; bias rides the ACT
                    # evacuation as a per-partition Identity bias
                    for co in range(4):
                        fps = ps_av.tile([128, 512], f32, tag="av")
                        for c in range(4):
                            nc.tensor.matmul(
                                fps[:, :Tm],
                                wp_sb[c][:, co * 128:(co + 1) * 128],
                                qkT[8 + c][:, :Tm],
                                start=(c == 0), stop=(c == 3))
                        yg = io.tile([128, 480], f32, tag="yg")
                        nc.scalar.activation(out=yg[:, :Tm], in_=fps[:, :Tm],
                                             func=Act.Identity,
                                             bias=beff_sb[:, co:co + 1])
                        nc.sync.dma_start(
                            out=y_d[co * 128:(co + 1) * 128, t0:t0 + Tm],
                            in_=yg[:, :Tm])
                else:
                    for gi in range(ng):
                        tg, go = tgs[gi], goffs[gi]
                        fps = ps_av.tile([TG, C], f32, tag="av")
                        for c in range(4):
                            nc.tensor.matmul(fps[:tg, :],
                                             qkT[8 + c][:, go:go + tg],
                                             wp_sb[c],
                                             start=(c == 0), stop=(c == 3))
                        yg = io.tile([TG, C], f32, tag="yg")
                        if cfg["ygp"]:
                            yr = io.tile([TG, C], f32, tag="yr")
                            nc.scalar.activation(out=yr[:tg, :],
                                                 in_=fps[:tg, :],
                                                 func=Act.Copy)
                            nc.gpsimd.tensor_tensor(out=yg[:tg, :],
                                                    in0=yr[:tg, :],
                                                    in1=beff_sb[:tg, :],
                                                    op=AluOpType.add)
                        else:
                            nc.vector.tensor_tensor(out=yg[:tg, :],
                                                    in0=fps[:tg, :],
                                                    in1=beff_sb[:tg, :],
                                                    op=AluOpType.add)
                        nc.sync.dma_start(out=y_d[t0 + go: t0 + go + tg, :],
                                          in_=yg[:tg, :])

    nc.compile()
    return nc


def _host_prep(x, W_qkv, b_qkv, outer, alpha, W_proj, b_proj):
    import ml_dtypes

    bf16 = ml_dtypes.bfloat16
    scale = np.float32(HD ** -0.5)
    x = np.ascontiguousarray(np.asarray(x, dtype=np.float32))
    W_qkv = np.asarray(W_qkv, dtype=np.float32)
    b_qkv = np.asarray(b_qkv, dtype=np.float32)
    outer = np.asarray(outer, dtype=np.float32)
    alpha = np.asarray(alpha, dtype=np.float32)
    W_proj = np.asarray(W_proj, dtype=np.float32)
    b_proj = np.asarray(b_proj, dtype=np.float32)

    xhi = x.astype(bf16)
    xlo = (x - xhi.astype(np.float32)).astype(bf16)

    wqk = np.concatenate([W_qkv[:, :C] * scale, W_qkv[:, C:2 * C]], axis=1)
    bqk = np.concatenate([b_qkv[:C] * scale, b_qkv[C:2 * C]])
    wv = W_qkv[:, 2 * C:]
    bv = b_qkv[2 * C:]
    beff = (b_proj + bv @ W_proj)[None, :]

    # multiplicative mask: 0 off-block, exp(alpha*outer*scale) on diag blocks
    base = np.zeros((H, TG, TG), dtype=np.float32)
    bias = np.exp((alpha[0] * scale) * outer)  # [H, 17, 17]
    for i in range(G):
        base[:, i * N:(i + 1) * N, i * N:(i + 1) * N] = bias
    mexp = base.astype(bf16)

    shared = {
        "bqk": np.ascontiguousarray(bqk),
        "beff": np.ascontiguousarray(beff),
        "mexp": np.ascontiguousarray(mexp),
    }
    if CFG.get("wlo", 0):
        for name, w in (("wqk", wqk), ("wv", wv), ("wp", W_proj)):
            wh = w.astype(bf16)
            wl = (w - wh.astype(np.float32)).astype(bf16)
            shared[name + "h"] = np.ascontiguousarray(wh)
            shared[name + "l"] = np.ascontiguousarray(wl)
    else:
        shared["wqk"] = np.ascontiguousarray(wqk)
        shared["wv"] = np.ascontiguousarray(wv)
        shared["wp"] = np.ascontiguousarray(W_proj)
    return xhi, xlo, shared


def kernel(x, W_qkv, b_qkv, outer, alpha, W_proj, b_proj, _trace=False):
    from concourse.bass_utils import run_bass_kernel_spmd

    if "nc" not in _CACHE:
        _CACHE["nc"] = _build_program(cfg=CFG)
    nc = _CACHE["nc"]

    xhi, xlo, shared = _host_prep(x, W_qkv, b_qkv, outer, alpha, W_proj, b_proj)
    use_dmat = CFG.get("dmat", 1)
    in_maps = []
    for c in range(NCORES):
        m = dict(shared)
        if use_dmat:
            m["xhi"] = np.ascontiguousarray(
                xhi[c * BC:(c + 1) * BC].reshape(NT, C))
            m["xlo"] = np.ascontiguousarray(
                xlo[c * BC:(c + 1) * BC].reshape(NT, C))
        else:
            m["x"] = np.ascontiguousarray(
                np.asarray(x, dtype=np.float32)[c * BC:(c + 1) * BC]
                .reshape(NT, C))
        in_maps.append(m)

    res = run_bass_kernel_spmd(nc, in_maps, core_ids=list(range(NCORES)),
                               trace=_trace)
    if CFG.get("yt", 1):
        out = np.concatenate(
            [np.ascontiguousarray(res.results[c]["y"].T).reshape(BC, N, C)
             for c in range(NCORES)], axis=0)
    else:
        out = np.concatenate(
            [res.results[c]["y"].reshape(BC, N, C) for c in range(NCORES)],
            axis=0)
    if _trace:
        _CACHE["last_result"] = res
    return out
